# revision 10
# baseline (speedup 1.0000x reference)
"""DTW kernel for Trainium2 (Bass/Tile).

Math (matches the reference):
  C[k,i] = max(0, |kernel_k|^2 + |x_i|^2 - 2 kernel_k . x_i)   (squared L2)
  D = DTW DP over C;  output = D[K-1, I-1]  (== path-sum of the reference
  backtrack to ~1e-6 rel, verified offline).

Device algorithm (single core, replicated on all 8 cores; the DP is a
serial recurrence so data-parallel sharding can't help and communication
latency would hurt):

  Phase A (PE):  C = Relu((-2 K^T).T @ X^T + x2 + k2) streamed to DRAM.
  Phase B (DVE): skewed-wavefront DP.  128 lanes; lane p owns columns
    [64p, 64p+64).  At step t lane p processes row (t-p) of its block:
      m[j] = min(D_prev[j-1], D_prev[j])
      e    = m + c
      D    = scan_j( state = min(e[j], state + c[j]) )   -- one DVE
             tensor_tensor_scan with initial = carry from lane p-1
    The carry (block-boundary D value) moves one lane per step via a
    PE shift-matmul (+BIG constant at lane 0 through a rank-1 accumulate).
  The skewed c tile for step t is one DMA with a diagonal DRAM access
  pattern (stride 64 - I per lane) from a row-padded copy of C.
"""

import os

import numpy as np

K_LEN, I_LEN, DIM = 1024, 8192, 128
LANES = 128
BW = I_LEN // LANES  # 64 columns per lane
PAD = LANES - 1
STEPS = K_LEN + LANES - 1
BIG = 1.0e30
XBW = 512  # gemm free-dim block (one PSUM bank of fp32)
N_CORES = 8


def _build_bass():
    import concourse.bass as bass
    import concourse.tile as tile
    from concourse import bacc, mybir
    from contextlib import ExitStack

    f32 = mybir.dt.float32
    Alu = mybir.AluOpType
    Act = mybir.ActivationFunctionType

    nc = bacc.Bacc("TRN2", target_bir_lowering=False, debug=False)

    k_in = nc.dram_tensor("kernel", [K_LEN, DIM], f32, kind="ExternalInput").ap()
    x_in = nc.dram_tensor("x", [I_LEN, DIM], f32, kind="ExternalInput").ap()
    ident_in = nc.dram_tensor("ident", [128, 128], f32, kind="ExternalInput").ap()
    shift_in = nc.dram_tensor("shiftmat", [128, 128], f32, kind="ExternalInput").ap()
    carry0_in = nc.dram_tensor("carry0", [128, 1], f32, kind="ExternalInput").ap()
    out_d = nc.dram_tensor("out", [LANES, 1], f32, kind="ExternalOutput").ap()
    cmat = nc.dram_tensor("cmat", [K_LEN + 2 * PAD, I_LEN], f32, kind="Internal").ap()

    KC = K_LEN // 128  # k chunks
    XCH = I_LEN // 128  # x chunks for transposes
    XB = I_LEN // XBW  # gemm column blocks

    with tile.TileContext(nc) as tc, ExitStack() as ctx:
        singles = ctx.enter_context(tc.tile_pool(name="singles", bufs=1))
        gps = ctx.enter_context(tc.tile_pool(name="gps", bufs=2, space="PSUM"))
        carry_ps = ctx.enter_context(tc.tile_pool(name="carry_ps", bufs=3, space="PSUM"))
        jnk = ctx.enter_context(tc.tile_pool(name="jnk", bufs=1, space="PSUM"))

        # ---- persistent tiles ----
        ident = singles.tile([128, 128], f32)
        shiftm = singles.tile([128, 128], f32)
        onescol = singles.tile([128, 1], f32)
        onesrow = singles.tile([1, 128], f32)
        KTm2 = singles.tile([128, K_LEN], f32)
        XT = singles.tile([128, I_LEN], f32)
        k2 = singles.tile([128, KC], f32)
        x2 = singles.tile([1, I_LEN], f32)
        D = singles.tile([128, BW], f32)
        shc0 = singles.tile([128, 1], f32)  # all BIG
        shcm1 = singles.tile([128, 1], f32)  # 0 at lane LANES-1, BIG elsewhere
        jps = jnk.tile([1, 1], f32)

        def pe_touch(t):
            # PE matmuls (LDWEIGHTS) accept only ONE sync wait in codegen.
            # This 1x1 junk matmul makes the PE clock observe `t`'s producer
            # so the real matmul that reads it needs one fewer wait.
            nc.tensor.matmul(jps, t[0:1, 0:1], t[0:1, 0:1], start=True, stop=True)

        nc.sync.dma_start(ident, ident_in)
        nc.sync.dma_start(shiftm, shift_in)
        nc.sync.dma_start(shcm1, carry0_in)
        pe_touch(ident)
        pe_touch(shiftm)
        nc.vector.memset(onescol, 1.0)
        nc.vector.memset(onesrow, 1.0)
        nc.vector.memset(D, BIG)
        nc.vector.memset(shc0, BIG)

        # ---- preamble: transposes, norms, cmat padding ----
        with tc.tile_pool(name="pre", bufs=3) as pre, tc.tile_pool(
            name="prez", bufs=1
        ) as prez:
            # fill cmat padding rows with BIG: garbage lanes must look like
            # +inf cost so no path can restart there, yet stay finite
            # (each garbage row adds ~1e30; <= 2*PAD rows < 3e32 << f32 max)
            bigz = prez.tile([128, 2048], f32)
            nc.vector.memset(bigz, BIG)
            zw = min(2048, I_LEN)
            for base_r in (0, PAD + K_LEN):
                for r0 in range(0, PAD, 128):
                    rows = min(128, PAD - r0)
                    for cb in range(0, I_LEN, zw):
                        nc.sync.dma_start(
                            cmat[base_r + r0 : base_r + r0 + rows, cb : cb + zw],
                            bigz[0:rows, 0:zw],
                        )

            # kernel chunks: natural load, k2, transpose, scale by -2
            for kc in range(KC):
                ksb = pre.tile([128, DIM], f32, tag="ksb")
                nc.sync.dma_start(ksb, k_in[kc * 128 : (kc + 1) * 128, :])
                ksq = pre.tile([128, DIM], f32, tag="ksq")
                nc.vector.tensor_mul(ksq, ksb, ksb)
                nc.vector.reduce_sum(k2[:, kc : kc + 1], ksq, axis=mybir.AxisListType.X)
                pe_touch(ksb)
                tps = gps.tile([128, 128], f32, tag="ps")
                nc.tensor.transpose(tps, ksb, ident)
                nc.scalar.mul(KTm2[:, kc * 128 : (kc + 1) * 128], tps, -2.0)

            # x chunks: natural load, transpose into XT
            for xc in range(XCH):
                xsb = pre.tile([128, DIM], f32, tag="xsb")
                nc.sync.dma_start(xsb, x_in[xc * 128 : (xc + 1) * 128, :])
                pe_touch(xsb)
                tps = gps.tile([128, 128], f32, tag="ps")
                nc.tensor.transpose(tps, xsb, ident)
                nc.scalar.copy(XT[:, xc * 128 : (xc + 1) * 128], tps)

            # x2 row: sum over d of XT^2, via ones-matmul on squared XT
            xt2 = prez.tile([128, I_LEN], f32)
            nc.vector.tensor_mul(xt2, XT, XT)
            pe_touch(xt2)
            for xb in range(XB):
                x2ps = gps.tile([1, XBW], f32, tag="ps")
                nc.tensor.matmul(
                    x2ps, onescol, xt2[:, xb * XBW : (xb + 1) * XBW], start=True, stop=True
                )
                nc.scalar.copy(x2[0:1, xb * XBW : (xb + 1) * XBW], x2ps)

        # ---- GEMM: C = Relu(-2 K X^T + x2 + k2) -> cmat rows ----
        with tc.tile_pool(name="stage", bufs=4) as stage_pool:
            for kc in range(KC):
                for xb in range(XB):
                    ps = gps.tile([128, XBW], f32, tag="ps")
                    nc.tensor.matmul(
                        ps,
                        KTm2[:, kc * 128 : (kc + 1) * 128],
                        XT[:, xb * XBW : (xb + 1) * XBW],
                        start=True,
                        stop=False,
                    )
                    nc.tensor.matmul(
                        ps,
                        onesrow,
                        x2[0:1, xb * XBW : (xb + 1) * XBW],
                        start=False,
                        stop=True,
                    )
                    st = stage_pool.tile([128, XBW], f32, tag="st")
                    nc.scalar.activation(
                        st, ps, Act.Relu, bias=k2[:, kc : kc + 1], scale=1.0
                    )
                    nc.sync.dma_start(
                        cmat[
                            PAD + kc * 128 : PAD + (kc + 1) * 128,
                            xb * XBW : (xb + 1) * XBW,
                        ],
                        st,
                    )

        # ---- DP: skewed wavefront ----
        cfl = cmat.rearrange("a b -> (a b)")
        with tc.tile_pool(name="cskew", bufs=16) as cpool, tc.tile_pool(
            name="dp", bufs=2
        ) as dp:
            ps_hist = [None, None]  # carry psum of steps t-1, t-2
            for t in range(STEPS):
                c_t = cpool.tile([128, BW], f32, tag="c")
                # lane p owns block (LANES-1-p); at step t it processes row
                # t-PAD+p, i.e. padded row t+p, cols [BW*(LANES-1-p), +BW).
                # Positive element stride I_LEN-BW per lane.
                src = bass.AP(
                    tensor=cfl.tensor,
                    offset=cfl.offset + t * I_LEN + (LANES - 1) * BW,
                    ap=[[I_LEN - BW, LANES], [1, BW]],
                )
                nc.sync.dma_start(c_t, src)

                m_t = dp.tile([128, BW], f32, tag="m")
                opb_src = shcm1 if t == 0 else (shc0 if t == 1 else ps_hist[1])
                init_src = shc0 if t == 0 else ps_hist[0]
                nc.vector.tensor_tensor(m_t[:, 0:1], D[:, 0:1], opb_src, op=Alu.min)
                nc.vector.tensor_tensor(
                    m_t[:, 1:BW], D[:, 0 : BW - 1], D[:, 1:BW], op=Alu.min
                )
                # D[j] = (m[j] min state) + c[j]  -- the DTW recurrence directly
                nc.vector.tensor_tensor_scan(
                    out=D,
                    data0=m_t,
                    data1=c_t,
                    initial=init_src,
                    op0=Alu.min,
                    op1=Alu.add,
                )
                if t < STEPS - 1:
                    cps = carry_ps.tile([128, 1], f32, tag="cps")
                    nc.tensor.matmul(cps, shiftm, D[:, BW - 1 : BW], start=True, stop=True)
                    ps_hist = [cps, ps_hist[0]]

        nc.sync.dma_start(out_d, D[:, BW - 1 : BW])

    nc.compile()
    return nc


_CACHE = {}


def _get_nc():
    if "nc" not in _CACHE:
        _CACHE["nc"] = _build_bass()
    return _CACHE["nc"]


def host_consts():
    ident = np.eye(128, dtype=np.float32)
    shiftmat = np.zeros((128, 128), np.float32)
    for p in range(127):
        shiftmat[p + 1, p] = 1.0  # out[p] = sum_q lhsT[q,p] v[q] = v[p+1]
    # column 127 (block 0's lane): huge positive dot with the (positive)
    # carry vector stands in for +inf (no second matmul needed)
    shiftmat[:, 127] = 1.0e24
    carry0 = np.full((128, 1), BIG, np.float32)
    carry0[127, 0] = 0.0  # virtual D[-1] for the (0,0) path start
    return {"ident": ident, "shiftmat": shiftmat, "carry0": carry0}


def run(kernel, x, trace=False, **kw):
    from concourse.bass_utils import run_bass_kernel_spmd

    nc = _get_nc()
    in_map = {
        "kernel": np.ascontiguousarray(kernel, dtype=np.float32),
        "x": np.ascontiguousarray(x, dtype=np.float32),
        **host_consts(),
    }
    res = run_bass_kernel_spmd(
        nc, [in_map] * N_CORES, core_ids=list(range(N_CORES)), trace=trace, **kw
    )
    return res


def kernel(kernel, x):
    res = run(kernel, x)
    col = res.results[0]["out"]
    return np.float32(col[0, 0])
